# revision 30
# baseline (speedup 1.0000x reference)
"""Causal self-attention (B=4, T=2048, C=1024, H=16) on 8 TRN2 NeuronCores.

Sharding: (batch b, head-group g) -> core 2*b+g. Each core computes, for its
batch and its 8 heads: qkv projection, causal attention, and a partial output
projection restricted to its heads' feature columns. Host sums the two
head-group partials per batch and adds the projection bias.

All device inputs are float16, pre-packed on the host into the exact SBUF
tile layouts so every load is one dense DMA (the sim's DMA engine is a
serial FIFO in program order; small/urgent transfers are issued first).

Device layouts (per core):
  xTd   [128, 8ck, 4qq, 512t]   x[b] chunked: partition p = channel ck*128+p
  wqk   [128, 8r, 8ck, 128m]    q|k weight chunks, r = output-feature block
  wv    [128, 8ck, 512f]        v weights
  wp    [128, 4ft, 1024c]       proj weight rows for this head group
  q/k are produced feature-major ([d, t]); scores S^T = kT.T @ qT come out
  [tk, tq] in PSUM, exp'd on ACT (the only ACT work), masked on DVE.
  AV runs flipped: psum[128 queries, 65] += Ew_block.T @ v_aug_block, so the
  moving dim is 65 (dv + denominator column) instead of 512 - half the PE
  cycles of the unflipped form. y is divided by the denominator (DVE), then
  PE-transposed back to feature-major for the output projection.
  QKV/proj matmul groups are interleaved into the attention loops at
  key-block granularity so the PE has filler work while ACT exps drain.
"""

from functools import partial

import numpy as np

N_CORES = 8
B, T, C, H, D = 4, 2048, 1024, 16, 64
F = 512          # features per head-group (8 heads x 64)
TQ = 512         # query block
TK = 128         # key block (psum partition dim)

_CACHE = {}


def _build_bass(debug=False):
    import sys
    if '/opt/trn_rl_repo' not in sys.path:
        sys.path.insert(0, '/opt/trn_rl_repo')
    import concourse.tile as tile
    from concourse import bacc, mybir

    f32 = mybir.dt.float32
    f16 = mybir.dt.float16
    AF = mybir.ActivationFunctionType

    nc = bacc.Bacc("TRN2", target_bir_lowering=False, debug=False,
                   num_devices=N_CORES)
    xTd = nc.dram_tensor("xTd", [128, 8, 4, 512], f16, kind="ExternalInput").ap()
    wqk = nc.dram_tensor("wqk", [128, 8, 8, 128], f16, kind="ExternalInput").ap()
    wv = nc.dram_tensor("wv", [128, 8, F], f16, kind="ExternalInput").ap()
    wp = nc.dram_tensor("wp", [128, 4, C], f16, kind="ExternalInput").ap()
    bqk = nc.dram_tensor("bqk", [128, 8], f32, kind="ExternalInput").ap()
    bvb = nc.dram_tensor("bvb", [128, F], f16, kind="ExternalInput").ap()
    masks = nc.dram_tensor("masks", [TK, 4 * TQ], f16, kind="ExternalInput").ap()
    ident = nc.dram_tensor("ident", [128, 128], f16, kind="ExternalInput").ap()
    part = nc.dram_tensor("part", [T, C], f16, kind="ExternalOutput").ap()
    if debug:
        d_qT = nc.dram_tensor("d_qT", [128, 4, T], f16,
                              kind="ExternalOutput").ap()
        d_kT = nc.dram_tensor("d_kT", [128, 4, T], f16,
                              kind="ExternalOutput").ap()
        d_v = nc.dram_tensor("d_v", [128, 16, 8, D + 1], f16,
                             kind="ExternalOutput").ap()
        d_yT = nc.dram_tensor("d_yT", [128, 4, T], f16,
                              kind="ExternalOutput").ap()
        d_av4 = nc.dram_tensor("d_av4", [128, 4, 128], f32,
                               kind="ExternalOutput").ap()
        d_Ew = nc.dram_tensor("d_Ew", [128, 1024], f16,
                              kind="ExternalOutput").ap()

    with tile.TileContext(nc) as tc:
        with (tc.tile_pool(name="singles", bufs=1) as S,
              tc.tile_pool(name="xq", bufs=3) as XQ,
              tc.tile_pool(name="ep", bufs=4) as EP,
              tc.tile_pool(name="small", bufs=2) as SM,
              tc.tile_pool(name="ob", bufs=4) as OB,
              tc.tile_pool(name="ps", bufs=2, space="PSUM") as PS):
            bqk_sb = S.tile([128, 8], f32, tag="bqk")
            ident_sb = S.tile([128, 128], f16, tag="ident")
            bvb_sb = S.tile([128, F], f16, tag="bvb")
            wqk_sb = S.tile([128, 8, 8, 128], f16, tag="wqk")
            wv_sb = S.tile([128, 8, F], f16, tag="wv")
            wp_sb = S.tile([128, 4, C], f16, tag="wp")
            mask_sb = S.tile([128, 4 * TQ], f16, tag="masks")
            qT = S.tile([128, 4, T], f16, tag="qT")
            kT = S.tile([128, 4, T], f16, tag="kT")
            v_aug = S.tile([128, 16, 8, D + 1], f16, tag="v_aug")
            yT = S.tile([128, 4, T], f16, tag="yT")

            # urgent/small first; first QKV matmul needs xq0 + wqk r=0 only
            nc.sync.dma_start(out=bqk_sb, in_=bqk)
            nc.sync.dma_start(out=ident_sb, in_=ident)
            nc.sync.dma_start(out=bvb_sb, in_=bvb)
            xq0 = XQ.tile([128, 8, 512], f16, tag="xq")
            # two DMAs: the first 4 ck chunks land sooner and the first
            # QKV matmuls start on them (subtile deps)
            nc.sync.dma_start(out=xq0[:, 0:4, :], in_=xTd[:, 0:4, 0, :])
            nc.sync.dma_start(out=xq0[:, 4:8, :], in_=xTd[:, 4:8, 0, :])
            for rj in range(8):
                nc.sync.dma_start(out=wqk_sb[:, rj], in_=wqk[:, rj])
            nc.sync.dma_start(out=wv_sb, in_=wv)
            nc.sync.dma_start(out=mask_sb, in_=masks)
            nc.sync.dma_start(out=wp_sb, in_=wp)

            nc.vector.memset(v_aug[:, :, :, D:D + 1], 1.0)

            def emit_qkv_unit(qq, xq, u):
                t0 = qq * 512
                if u < 8:
                    r = u
                    ps = PS.tile([128, 512], f32, tag="bp")
                    for ck in range(8):
                        nc.tensor.matmul(ps, wqk_sb[:, r, ck, :], xq[:, ck, :],
                                         start=(ck == 0), stop=(ck == 7))
                    dest = qT if r < 4 else kT
                    nc.vector.tensor_scalar_add(
                        out=dest[:, r % 4, t0:t0 + 512], in0=ps,
                        scalar1=bqk_sb[:, r:r + 1])
                else:
                    tt = u - 8
                    psv = PS.tile([128, 512], f32, tag="bp")
                    for ck in range(8):
                        nc.tensor.matmul(psv, xq[:, ck, 128 * tt:128 * tt + 128],
                                         wv_sb[:, ck, :],
                                         start=(ck == 0), stop=(ck == 7))
                    nc.vector.tensor_add(
                        out=v_aug[:, 4 * qq + tt, :, 0:D],
                        in0=psv.rearrange("p (h d) -> p h d", h=8),
                        in1=bvb_sb.rearrange("p (h d) -> p h d", h=8))

            proj_state = {}

            def emit_proj_unit(tqb, u):
                tt, jh = divmod(u, 2)
                t = 512 * tqb + 128 * tt
                if jh == 0:
                    proj_state[tqb] = OB.tile([128, 2, 512], f16, tag="ob",
                                              name="outsb")
                outsb = proj_state[tqb]
                pso = PS.tile([128, 512], f32, tag="bp")
                for ft in range(4):
                    nc.tensor.matmul(pso, yT[:, ft, t:t + 128],
                                     wp_sb[:, ft, 512 * jh:512 * jh + 512],
                                     start=(ft == 0), stop=(ft == 3))
                nc.vector.tensor_copy(out=outsb[:, jh, :], in_=pso)
                if jh == 1:
                    nc.sync.dma_start(
                        out=part[t:t + 128, :],
                        in_=outsb.rearrange("p a b -> p (a b)"))

            def emit_att(tqb, fillers, early=0):
                # Flat (head, kp) stream with AV matmuls trailing the
                # scores/exp by one tile, so the PE issues the next head's
                # first scores while ACT drains the current head's exps
                # (kills the ~2us ACT bubble at every head boundary).
                n_tkb = 4 * (tqb + 1)
                n_kp = n_tkb // 2
                q0 = TQ * tqb
                steps = [(h, kp) for h in range(8) for kp in range(n_kp)]
                state = {"fi": 0, "ui": 0}
                av4s = {}

                def pace():
                    state["ui"] += 1
                    want = max(min(2 * state["ui"], early),
                               len(fillers) * state["ui"] // len(steps))
                    while state["fi"] < min(want, len(fillers)):
                        fillers[state["fi"]]()
                        state["fi"] += 1

                def head_finish(av4, h):
                    hp, par = h // 2, h % 2
                    if debug and tqb == 0 and h == 0:
                        av_sb = EP.tile([128, 512], f32, tag="avdbg",
                                        name="av_sb", bufs=1)
                        nc.vector.tensor_copy(
                            out=av_sb,
                            in_=av4.rearrange("p a b -> p (a b)"))
                        nc.sync.dma_start(
                            out=d_av4,
                            in_=av_sb.rearrange("p (a b) -> p a b", a=4))
                    rq4 = SM.tile([128, 4, 1], f32, tag="rq4")
                    yq4 = SM.tile([128, 4, D], f16, tag="yq4")
                    # this DVE chain gates the PE transposes; don't let it
                    # queue behind filler bias-adds
                    with tc.high_priority():
                        nc.vector.reciprocal(out=rq4, in_=av4[:, :, D:D + 1])
                        for qc in range(4):
                            nc.vector.tensor_scalar_mul(
                                out=yq4[:, qc, :], in0=av4[:, qc, 0:D],
                                scalar1=rq4[:, qc, :])
                    bpT = PS.tile([128, 512], f32, tag="bp",
                                  name="bpT").bitcast(f16)
                    for qc in range(4):
                        nc.tensor.matmul(
                            bpT[0:D, 128 * qc:128 * qc + 128],
                            yq4[:, qc, :], ident_sb, is_transpose=True,
                            start=(qc == 0), stop=(qc == 3),
                            skip_group_check=True)
                    nc.vector.tensor_copy(
                        out=yT[64 * par:64 * par + 64, hp, q0:q0 + TQ],
                        in_=bpT[0:D, 0:TQ])

                def emit_scores(h, kp):
                    hp, par = h // 2, h % 2
                    ps2 = PS.tile([128, 1024], f32, tag="qk")
                    # scores feed the ACT exp stream (the attention
                    # bottleneck): highest scheduler priority so filler
                    # matmuls never delay them (the qk ring bounds run-ahead)
                    with tc.high_priority():
                        for half in range(2):
                            tkb = 2 * kp + half
                            d = tkb - 4 * tqb
                            c0 = 128 * d if d > 0 else 0
                            nc.tensor.matmul(
                                ps2[:, 512 * half + c0:512 * half + 512],
                                kT[64 * par:64 * par + 64, hp,
                                   TK * tkb:TK * tkb + TK],
                                qT[64 * par:64 * par + 64, hp,
                                   q0 + c0:q0 + TQ],
                                start=True, stop=True)
                    Ew = EP.tile([128, 1024], f16, tag="E")
                    d0 = 2 * kp - 4 * tqb
                    e0 = 128 * d0 if d0 > 0 else 0
                    nc.scalar.activation(out=Ew[:, e0:], in_=ps2[:, e0:],
                                         func=AF.Exp, scale=0.125)
                    if d0 >= 0:
                        with tc.high_priority():
                            nc.vector.tensor_mul(
                                out=Ew[:, e0:], in0=Ew[:, e0:],
                                in1=mask_sb[:, 512 * d0 + e0:512 * d0 + 1024])
                    if debug and tqb == 0 and h == 0 and kp == 0:
                        nc.sync.dma_start(out=d_Ew, in_=Ew)
                    return Ew

                def emit_avs(h, kp, Ew):
                    # One accumulation group per av4 bank: start=True arms
                    # the whole 2KB zero region, so only the first matmul
                    # starts; each qc's first touch overwrites via
                    # pending-zero, later ones accumulate.
                    if kp == 0:
                        av4s[h] = PS.tile([128, 4, 128], f32, tag="av4",
                                          name="av4")
                    av4 = av4s[h]
                    for half in range(2):
                        tkb = 2 * kp + half
                        for qc in range(4):
                            gqc = 4 * tqb + qc
                            if tkb > gqc:
                                continue
                            nc.tensor.matmul(
                                av4[:, qc, 0:D + 1],
                                Ew[:, 512 * half + 128 * qc:
                                   512 * half + 128 * qc + 128],
                                v_aug[:, tkb, h, :],
                                start=(tkb == 0 and qc == 0),
                                stop=(kp == n_kp - 1 and half == 1
                                      and qc == 3),
                                skip_group_check=True)
                    if kp == n_kp - 1:
                        head_finish(av4s.pop(h), h)

                trailing = []
                for h, kp in steps:
                    Ew = emit_scores(h, kp)
                    pace()
                    trailing.append((h, kp, Ew))
                    if len(trailing) > 1:
                        emit_avs(*trailing.pop(0))
                while trailing:
                    emit_avs(*trailing.pop(0))
                while state["fi"] < len(fillers):
                    fillers[state["fi"]]()
                    state["fi"] += 1

            # q/k units of block 0 only; its v units become att0's first
            # fillers so scores/exp start ~7us sooner
            for u in range(8):
                emit_qkv_unit(0, xq0, u)
            for tqb in range(4):
                fillers = []
                early = 0
                if tqb == 0:
                    fillers += [partial(emit_qkv_unit, 0, xq0, u)
                                for u in range(8, 12)]
                    early = 4
                if tqb < 3:
                    xqn = XQ.tile([128, 8, 512], f16, tag="xq")
                    nc.sync.dma_start(out=xqn, in_=xTd[:, :, tqb + 1, :])
                    fillers += [partial(emit_qkv_unit, tqb + 1, xqn, u)
                                for u in range(12)]
                if tqb == 3:
                    # back-load proj work into the exp-heaviest block
                    for pt in range(3):
                        fillers += [partial(emit_proj_unit, pt, u)
                                    for u in range(8)]
                emit_att(tqb, fillers, early=early)
            for u in range(8):
                emit_proj_unit(3, u)

            if debug:
                nc.sync.dma_start(out=d_qT, in_=qT)
                nc.sync.dma_start(out=d_kT, in_=kT)
                nc.sync.dma_start(out=d_v, in_=v_aug)
                nc.sync.dma_start(out=d_yT, in_=yT)

    nc.compile()
    return nc


def _get_nc():
    if "nc" not in _CACHE:
        _CACHE["nc"] = _build_bass()
    return _CACHE["nc"]


def _make_in_maps(x, W_attn, b_attn, W_proj):
    x = np.asarray(x, dtype=np.float32)
    W_attn = np.asarray(W_attn, dtype=np.float32)
    b_attn = np.asarray(b_attn, dtype=np.float32)
    W_proj = np.asarray(W_proj, dtype=np.float32)

    jj = np.arange(TQ)[None, :]
    ii = np.arange(TK)[:, None]
    # Staircase mask for the diagonal key-quad: column block hh (of 4)
    # holds key block (4*tqb + hh); valid iff local j >= 128*hh + i.
    masks = np.concatenate([(jj >= 128 * hh + ii) for hh in range(4)],
                           axis=1).astype(np.float16)
    ident = np.eye(128, dtype=np.float16)

    in_maps = []
    for c in range(N_CORES):
        b, g = divmod(c, 2)
        wq = W_attn[F * g:F * g + F]
        wk = W_attn[C + F * g:C + F * g + F]
        wv_ = W_attn[2 * C + F * g:2 * C + F * g + F]
        wqk_cat = np.concatenate([wq, wk], axis=0)          # [1024 f, 1024 c]
        # [p, r, ck, m] = wqk_cat[128r + m, 128ck + p]
        wqk_dev = wqk_cat.reshape(8, 128, 8, 128).transpose(3, 0, 2, 1)
        # [p, ck, f] = wv_[f, 128ck + p]
        wv_dev = wv_.reshape(F, 8, 128).transpose(2, 1, 0)
        mproj = W_proj[:, F * g:F * g + F].T                # [512 f, 1024 c]
        # [p, ft, c] = mproj[128ft + p, c]
        wp_dev = mproj.reshape(4, 128, C).transpose(1, 0, 2)
        # [p, ck, qq, t] = x[b, 512qq + t, 128ck + p]
        x_dev = x[b].reshape(4, 512, 8, 128).transpose(3, 2, 0, 1)
        bqk_flat = np.concatenate([b_attn[F * g:F * g + F],
                                   b_attn[C + F * g:C + F * g + F]])
        bv = b_attn[2 * C + F * g:2 * C + F * g + F]
        in_maps.append({
            "xTd": np.ascontiguousarray(x_dev, dtype=np.float16),
            "wqk": np.ascontiguousarray(wqk_dev, dtype=np.float16),
            "wv": np.ascontiguousarray(wv_dev, dtype=np.float16),
            "wp": np.ascontiguousarray(wp_dev, dtype=np.float16),
            "bqk": np.ascontiguousarray(bqk_flat.reshape(8, 128).T,
                                        dtype=np.float32),
            "bvb": np.ascontiguousarray(
                np.broadcast_to(bv[None, :], (128, F)), dtype=np.float16),
            "masks": masks,
            "ident": ident,
        })
    return in_maps


def kernel(x, W_attn, b_attn, W_proj, b_proj):
    import sys
    if '/opt/trn_rl_repo' not in sys.path:
        sys.path.insert(0, '/opt/trn_rl_repo')
    from concourse.bass_utils import run_bass_kernel_spmd

    nc = _get_nc()
    in_maps = _make_in_maps(x, W_attn, b_attn, W_proj)
    res = run_bass_kernel_spmd(nc, in_maps, core_ids=list(range(N_CORES)))
    b_proj = np.asarray(b_proj, dtype=np.float32)
    out = np.empty((B, T, C), dtype=np.float32)
    for b in range(B):
        out[b] = (res.results[2 * b]["part"].astype(np.float32)
                  + res.results[2 * b + 1]["part"].astype(np.float32)
                  + b_proj[None, :])
    return out


# revision 37
# speedup vs baseline: 1.0731x; 1.0731x over previous
"""Causal self-attention (B=4, T=2048, C=1024, H=16) on 8 TRN2 NeuronCores.

Sharding: (batch b, head-group g) -> core 2*b+g. Each core computes, for its
batch and its 8 heads: qkv projection, causal attention, and a partial output
projection restricted to its heads' feature columns. Host sums the two
head-group partials per batch and adds the projection bias.

All device inputs are float16, pre-packed on the host into the exact SBUF
tile layouts so every load is one dense DMA (the sim's DMA engine is a
serial FIFO in program order; small/urgent transfers are issued first).

Device layouts (per core):
  xTd   [128, 8ck, 4qq, 512t]   x[b] chunked: partition p = channel ck*128+p
  wqk   [128, 8r, 8ck, 128m]    q|k weight chunks, r = output-feature block
  wv    [128, 8ck, 512f]        v weights
  wp    [128, 4ft, 1024c]       proj weight rows for this head group
  q/k are produced feature-major ([d, t]); scores S^T = kT.T @ qT come out
  [tk, tq] in PSUM, exp'd on ACT (the only ACT work), masked on DVE.
  AV runs flipped: psum[128 queries, 65] += Ew_block.T @ v_aug_block, so the
  moving dim is 65 (dv + denominator column) instead of 512 - half the PE
  cycles of the unflipped form. y is divided by the denominator (DVE), then
  PE-transposed back to feature-major for the output projection.
  QKV/proj matmul groups are interleaved into the attention loops at
  key-block granularity so the PE has filler work while ACT exps drain.
"""

from functools import partial

import numpy as np

N_CORES = 8
B, T, C, H, D = 4, 2048, 1024, 16, 64
F = 512          # features per head-group (8 heads x 64)
TQ = 512         # query block
TK = 128         # key block (psum partition dim)

_CACHE = {}


def _build_bass(debug=False):
    import sys
    if '/opt/trn_rl_repo' not in sys.path:
        sys.path.insert(0, '/opt/trn_rl_repo')
    import concourse.tile as tile
    from concourse import bacc, mybir

    f32 = mybir.dt.float32
    f16 = mybir.dt.float16
    f8 = mybir.dt.float8e4
    AF = mybir.ActivationFunctionType
    DR = mybir.MatmulPerfMode.DoubleRow

    nc = bacc.Bacc("TRN2", target_bir_lowering=False, debug=False,
                   num_devices=N_CORES)
    xTd = nc.dram_tensor("xTd", [128, 8, 4, 512], f16, kind="ExternalInput").ap()
    wqk = nc.dram_tensor("wqk", [128, 8, 8, 128], f16, kind="ExternalInput").ap()
    wv = nc.dram_tensor("wv", [128, 8, F], f16, kind="ExternalInput").ap()
    wp = nc.dram_tensor("wp", [128, 4, C], f16, kind="ExternalInput").ap()
    bqk = nc.dram_tensor("bqk", [128, 8], f32, kind="ExternalInput").ap()
    bvb = nc.dram_tensor("bvb", [128, F], f16, kind="ExternalInput").ap()
    masks = nc.dram_tensor("masks", [TK, 4 * TQ], f16, kind="ExternalInput").ap()
    ident = nc.dram_tensor("ident", [128, 128], f16, kind="ExternalInput").ap()
    part = nc.dram_tensor("part", [T, C], f16, kind="ExternalOutput").ap()
    if debug:
        d_qT = nc.dram_tensor("d_qT", [128, 4, T], mybir.dt.float8e4,
                              kind="ExternalOutput").ap()
        d_kT = nc.dram_tensor("d_kT", [128, 4, T], mybir.dt.float8e4,
                              kind="ExternalOutput").ap()
        d_v = nc.dram_tensor("d_v", [128, 16, 8, D + 1], f16,
                             kind="ExternalOutput").ap()
        d_yT = nc.dram_tensor("d_yT", [128, 4, T], f16,
                              kind="ExternalOutput").ap()
        d_av4 = nc.dram_tensor("d_av4", [128, 4, 128], f32,
                               kind="ExternalOutput").ap()
        d_Ew = nc.dram_tensor("d_Ew", [128, 1024], f16,
                              kind="ExternalOutput").ap()

    with tile.TileContext(nc) as tc:
        with (tc.tile_pool(name="singles", bufs=1) as S,
              tc.tile_pool(name="xq", bufs=3) as XQ,
              tc.tile_pool(name="ep", bufs=4) as EP,
              tc.tile_pool(name="small", bufs=2) as SM,
              tc.tile_pool(name="ob", bufs=4) as OB,
              tc.tile_pool(name="ps", bufs=2, space="PSUM") as PS):
            bqk_sb = S.tile([128, 8], f32, tag="bqk")
            ident_sb = S.tile([128, 128], f16, tag="ident")
            bvb_sb = S.tile([128, F], f16, tag="bvb")
            wqk_sb = S.tile([128, 8, 8, 128], f16, tag="wqk")
            wv_sb = S.tile([128, 8, F], f16, tag="wv")
            wp_sb = S.tile([128, 4, C], f16, tag="wp")
            mask_sb = S.tile([128, 4 * TQ], f16, tag="masks")
            # q/k in fp8e4: scores run in DoubleRow perf mode (0.5
            # cycles/row, 2x PE throughput). The d=64 contraction packs as
            # 32 partitions x 2 interleaved rows (d = 32*i + p), produced
            # from the feature-major fp8 copies by an SBUF->SBUF DMA shuffle.
            qT = S.tile([128, 4, T], f8, tag="qT")
            kT = S.tile([128, 4, T], f8, tag="kT")
            qT8 = S.tile([32, 2, 8, T], f8, tag="qT8")
            kT8 = S.tile([32, 2, 8, T], f8, tag="kT8")
            v_aug = S.tile([128, 16, 8, D + 1], f16, tag="v_aug")
            yT = S.tile([128, 4, T], f16, tag="yT")

            # urgent/small first; first QKV matmul needs xq0 + wqk r=0 only
            nc.sync.dma_start(out=bqk_sb, in_=bqk)
            nc.sync.dma_start(out=ident_sb, in_=ident)
            nc.sync.dma_start(out=bvb_sb, in_=bvb)
            xq0 = XQ.tile([128, 8, 512], f16, tag="xq")
            # two DMAs: the first 4 ck chunks land sooner and the first
            # QKV matmuls start on them (subtile deps)
            nc.sync.dma_start(out=xq0[:, 0:4, :], in_=xTd[:, 0:4, 0, :])
            nc.sync.dma_start(out=xq0[:, 4:8, :], in_=xTd[:, 4:8, 0, :])
            for rj in range(8):
                nc.sync.dma_start(out=wqk_sb[:, rj], in_=wqk[:, rj])
            nc.sync.dma_start(out=wv_sb, in_=wv)
            nc.sync.dma_start(out=mask_sb, in_=masks)
            nc.sync.dma_start(out=wp_sb, in_=wp)

            nc.vector.memset(v_aug[:, :, :, D:D + 1], 1.0)

            def emit_qkv_unit(qq, xq, u):
                t0 = qq * 512
                if u < 8:
                    r = u
                    ps = PS.tile([128, 512], f32, tag="bp")
                    for ck in range(8):
                        nc.tensor.matmul(ps, wqk_sb[:, r, ck, :], xq[:, ck, :],
                                         start=(ck == 0), stop=(ck == 7))
                    dest = qT if r < 4 else kT
                    nc.vector.tensor_scalar_add(
                        out=dest[:, r % 4, t0:t0 + 512], in0=ps,
                        scalar1=bqk_sb[:, r:r + 1])
                    if r == 3 or r == 7:
                        # repack fp8 q/k into the DoubleRow pair layout:
                        # dst[p, i, h=2r+a, t] = src[64a + 32i + p, r, t]
                        # (d = 32i + p). 4 DMAs keep each AP at 3 dims.
                        src, dst = (qT, qT8) if r == 3 else (kT, kT8)
                        for a in range(2):
                            for i in range(2):
                                p0 = 64 * a + 32 * i
                                nc.sync.dma_start(
                                    out=dst[:, i, a:8:2, t0:t0 + 512],
                                    in_=src[p0:p0 + 32, :, t0:t0 + 512])
                else:
                    tt = u - 8
                    psv = PS.tile([128, 512], f32, tag="bp")
                    for ck in range(8):
                        nc.tensor.matmul(psv, xq[:, ck, 128 * tt:128 * tt + 128],
                                         wv_sb[:, ck, :],
                                         start=(ck == 0), stop=(ck == 7))
                    nc.vector.tensor_add(
                        out=v_aug[:, 4 * qq + tt, :, 0:D],
                        in0=psv.rearrange("p (h d) -> p h d", h=8),
                        in1=bvb_sb.rearrange("p (h d) -> p h d", h=8))

            proj_state = {}

            def emit_proj_unit(tqb, u):
                tt, jh = divmod(u, 2)
                t = 512 * tqb + 128 * tt
                if jh == 0:
                    proj_state[tqb] = OB.tile([128, 2, 512], f16, tag="ob",
                                              name="outsb")
                outsb = proj_state[tqb]
                pso = PS.tile([128, 512], f32, tag="bp")
                for ft in range(4):
                    nc.tensor.matmul(pso, yT[:, ft, t:t + 128],
                                     wp_sb[:, ft, 512 * jh:512 * jh + 512],
                                     start=(ft == 0), stop=(ft == 3))
                nc.vector.tensor_copy(out=outsb[:, jh, :], in_=pso)
                if jh == 1:
                    nc.sync.dma_start(
                        out=part[t:t + 128, :],
                        in_=outsb.rearrange("p a b -> p (a b)"))

            def emit_att(tqb, fillers, early=0):
                # Flat (head, kp) stream with AV matmuls trailing the
                # scores/exp by one tile, so the PE issues the next head's
                # first scores while ACT drains the current head's exps
                # (kills the ~2us ACT bubble at every head boundary).
                n_tkb = 4 * (tqb + 1)
                n_kp = n_tkb // 2
                q0 = TQ * tqb
                steps = [(h, kp) for h in range(8) for kp in range(n_kp)]
                state = {"fi": 0, "ui": 0}
                av4s = {}

                def pace():
                    state["ui"] += 1
                    want = max(min(2 * state["ui"], early),
                               len(fillers) * state["ui"] // len(steps))
                    while state["fi"] < min(want, len(fillers)):
                        fillers[state["fi"]]()
                        state["fi"] += 1

                def head_finish(av4, h):
                    hp, par = h // 2, h % 2
                    if debug and tqb == 0 and h == 0:
                        av_sb = EP.tile([128, 512], f32, tag="avdbg",
                                        name="av_sb", bufs=1)
                        nc.vector.tensor_copy(
                            out=av_sb,
                            in_=av4.rearrange("p a b -> p (a b)"))
                        nc.sync.dma_start(
                            out=d_av4,
                            in_=av_sb.rearrange("p (a b) -> p a b", a=4))
                    rq4 = SM.tile([128, 4, 1], f32, tag="rq4")
                    nc.vector.reciprocal(out=rq4, in_=av4[:, :, D:D + 1])
                    yq4 = SM.tile([128, 4, D], f16, tag="yq4")
                    for qc in range(4):
                        nc.vector.tensor_scalar_mul(
                            out=yq4[:, qc, :], in0=av4[:, qc, 0:D],
                            scalar1=rq4[:, qc, :])
                    bpT = PS.tile([128, 512], f32, tag="bp",
                                  name="bpT").bitcast(f16)
                    for qc in range(4):
                        nc.tensor.matmul(
                            bpT[0:D, 128 * qc:128 * qc + 128],
                            yq4[:, qc, :], ident_sb, is_transpose=True,
                            start=(qc == 0), stop=(qc == 3),
                            skip_group_check=True)
                    nc.vector.tensor_copy(
                        out=yT[64 * par:64 * par + 64, hp, q0:q0 + TQ],
                        in_=bpT[0:D, 0:TQ])

                def emit_scores(h, kp):
                    hp, par = h // 2, h % 2
                    ps2 = PS.tile([128, 1024], f32, tag="qk")
                    # scores feed the ACT exp stream (the attention
                    # bottleneck): highest scheduler priority so filler
                    # matmuls never delay them (the qk ring bounds run-ahead)
                    with tc.high_priority():
                        for half in range(2):
                            tkb = 2 * kp + half
                            d = tkb - 4 * tqb
                            c0 = 128 * d if d > 0 else 0
                            nc.tensor.matmul(
                                ps2[:, 512 * half + c0:512 * half + 512],
                                kT8[:, :, h, TK * tkb:TK * tkb + TK],
                                qT8[:, :, h, q0 + c0:q0 + TQ],
                                start=True, stop=True, perf_mode=DR)
                    Ew = EP.tile([128, 1024], f16, tag="E")
                    d0 = 2 * kp - 4 * tqb
                    e0 = 128 * d0 if d0 > 0 else 0
                    nc.scalar.activation(out=Ew[:, e0:], in_=ps2[:, e0:],
                                         func=AF.Exp, scale=0.125)
                    if d0 >= 0:
                        with tc.high_priority():
                            nc.vector.tensor_mul(
                                out=Ew[:, e0:], in0=Ew[:, e0:],
                                in1=mask_sb[:, 512 * d0 + e0:512 * d0 + 1024])
                    if debug and tqb == 0 and h == 0 and kp == 0:
                        nc.sync.dma_start(out=d_Ew, in_=Ew)
                    return Ew

                def emit_avs(h, kp, Ew):
                    # One accumulation group per av4 bank: start=True arms
                    # the whole 2KB zero region, so only the first matmul
                    # starts; each qc's first touch overwrites via
                    # pending-zero, later ones accumulate.
                    if kp == 0:
                        av4s[h] = PS.tile([128, 4, 128], f32, tag="av4",
                                          name="av4")
                    av4 = av4s[h]
                    for half in range(2):
                        tkb = 2 * kp + half
                        for qc in range(4):
                            gqc = 4 * tqb + qc
                            if tkb > gqc:
                                continue
                            nc.tensor.matmul(
                                av4[:, qc, 0:D + 1],
                                Ew[:, 512 * half + 128 * qc:
                                   512 * half + 128 * qc + 128],
                                v_aug[:, tkb, h, :],
                                start=(tkb == 0 and qc == 0),
                                stop=(kp == n_kp - 1 and half == 1
                                      and qc == 3),
                                skip_group_check=True)
                    if kp == n_kp - 1:
                        head_finish(av4s.pop(h), h)

                trailing = []
                for h, kp in steps:
                    Ew = emit_scores(h, kp)
                    pace()
                    trailing.append((h, kp, Ew))
                    if len(trailing) > 1:
                        emit_avs(*trailing.pop(0))
                while trailing:
                    emit_avs(*trailing.pop(0))
                while state["fi"] < len(fillers):
                    fillers[state["fi"]]()
                    state["fi"] += 1

            # q/k units of block 0 only; its v units become att0's first
            # fillers so scores/exp start ~7us sooner
            for u in range(8):
                emit_qkv_unit(0, xq0, u)
            for tqb in range(4):
                fillers = []
                early = 0
                if tqb == 0:
                    fillers += [partial(emit_qkv_unit, 0, xq0, u)
                                for u in range(8, 12)]
                    early = 4
                if tqb < 3:
                    xqn = XQ.tile([128, 8, 512], f16, tag="xq")
                    nc.sync.dma_start(out=xqn, in_=xTd[:, :, tqb + 1, :])
                    fillers += [partial(emit_qkv_unit, tqb + 1, xqn, u)
                                for u in range(12)]
                if tqb == 3:
                    # back-load proj work into the exp-heaviest block
                    for pt in range(3):
                        fillers += [partial(emit_proj_unit, pt, u)
                                    for u in range(8)]
                emit_att(tqb, fillers, early=early)
            for u in range(8):
                emit_proj_unit(3, u)

            if debug:
                nc.sync.dma_start(out=d_qT, in_=qT)
                nc.sync.dma_start(out=d_kT, in_=kT)
                nc.sync.dma_start(out=d_v, in_=v_aug)
                nc.sync.dma_start(out=d_yT, in_=yT)

    nc.compile()
    return nc


def _get_nc():
    if "nc" not in _CACHE:
        _CACHE["nc"] = _build_bass()
    return _CACHE["nc"]


def _make_in_maps(x, W_attn, b_attn, W_proj):
    x = np.asarray(x, dtype=np.float32)
    W_attn = np.asarray(W_attn, dtype=np.float32)
    b_attn = np.asarray(b_attn, dtype=np.float32)
    W_proj = np.asarray(W_proj, dtype=np.float32)

    jj = np.arange(TQ)[None, :]
    ii = np.arange(TK)[:, None]
    # Staircase mask for the diagonal key-quad: column block hh (of 4)
    # holds key block (4*tqb + hh); valid iff local j >= 128*hh + i.
    masks = np.concatenate([(jj >= 128 * hh + ii) for hh in range(4)],
                           axis=1).astype(np.float16)
    ident = np.eye(128, dtype=np.float16)

    in_maps = []
    for c in range(N_CORES):
        b, g = divmod(c, 2)
        wq = W_attn[F * g:F * g + F]
        wk = W_attn[C + F * g:C + F * g + F]
        wv_ = W_attn[2 * C + F * g:2 * C + F * g + F]
        wqk_cat = np.concatenate([wq, wk], axis=0)          # [1024 f, 1024 c]
        # [p, r, ck, m] = wqk_cat[128r + m, 128ck + p]
        wqk_dev = wqk_cat.reshape(8, 128, 8, 128).transpose(3, 0, 2, 1)
        # [p, ck, f] = wv_[f, 128ck + p]
        wv_dev = wv_.reshape(F, 8, 128).transpose(2, 1, 0)
        mproj = W_proj[:, F * g:F * g + F].T                # [512 f, 1024 c]
        # [p, ft, c] = mproj[128ft + p, c]
        wp_dev = mproj.reshape(4, 128, C).transpose(1, 0, 2)
        # [p, ck, qq, t] = x[b, 512qq + t, 128ck + p]
        x_dev = x[b].reshape(4, 512, 8, 128).transpose(3, 2, 0, 1)
        bqk_flat = np.concatenate([b_attn[F * g:F * g + F],
                                   b_attn[C + F * g:C + F * g + F]])
        bv = b_attn[2 * C + F * g:2 * C + F * g + F]
        in_maps.append({
            "xTd": np.ascontiguousarray(x_dev, dtype=np.float16),
            "wqk": np.ascontiguousarray(wqk_dev, dtype=np.float16),
            "wv": np.ascontiguousarray(wv_dev, dtype=np.float16),
            "wp": np.ascontiguousarray(wp_dev, dtype=np.float16),
            "bqk": np.ascontiguousarray(bqk_flat.reshape(8, 128).T,
                                        dtype=np.float32),
            "bvb": np.ascontiguousarray(
                np.broadcast_to(bv[None, :], (128, F)), dtype=np.float16),
            "masks": masks,
            "ident": ident,
        })
    return in_maps


def kernel(x, W_attn, b_attn, W_proj, b_proj):
    import sys
    if '/opt/trn_rl_repo' not in sys.path:
        sys.path.insert(0, '/opt/trn_rl_repo')
    from concourse.bass_utils import run_bass_kernel_spmd

    nc = _get_nc()
    in_maps = _make_in_maps(x, W_attn, b_attn, W_proj)
    res = run_bass_kernel_spmd(nc, in_maps, core_ids=list(range(N_CORES)))
    b_proj = np.asarray(b_proj, dtype=np.float32)
    out = np.empty((B, T, C), dtype=np.float32)
    for b in range(B):
        out[b] = (res.results[2 * b]["part"].astype(np.float32)
                  + res.results[2 * b + 1]["part"].astype(np.float32)
                  + b_proj[None, :])
    return out
